# revision 4
# baseline (speedup 1.0000x reference)
"""BiLSTM Trainium2 kernel v2 (8 NeuronCores, SPMD).

Problem: inputs [64, 512, 256] f32, BiLSTM hidden 512, out = (fwd + bwd)/2.

Sharding: 24 units = 2 dirs x 2 batch-shards(32) x 6 seq-chunks; 3 units
("streams") per core, interleaved so each stream's recurrence-chain latency
hides behind the other streams' PE work. Seq chunks use a 20-step warmup
(LSTM state relaxes from zero; influence of truncated history decays
~prod(f) ~ e^-15 over 20 steps, far below the 2e-2 tolerance). Chunk 0
needs no warmup, so chunk lengths are 102/82/.../82 and every unit runs
exactly T=102 steps (SPMD-uniform).

Per-core per-stream step (batch 32, no M-padding):
  gates PSUM bank [128p, 512f]: partition 32j+b (j = h-block, b = batch),
  free 128g+k (g = gate f/i/o/C, k = h-dim within block).
  PE: bias inject (K=1, 4 col-tiled MMs) + x@Wx (2 K-chunks) + h@Wh
      (4 K-chunks), each chunk = 4 col-tiled MMs N=512 (full 128x128 array).
  ACT: ONE sigmoid over all 512 cols (C-gate weights pre-scaled x2;
      tanh(g) = 2*sigmoid(2g)-1 folded into DVE scalar_tensor_tensor).
  DVE: u=(sg2-0.5)*si; c'=2u+m; h=so*tanh(c'); m=sf*c; hT copy. fp16 cell.
  PE: transpose h -> hT (fp16) for next step's stationary.
Weights column-permuted: new col 512*j + 128*g + k <- orig 512*g + 128*j + k,
then C-gate (g=3) columns scaled x2.
"""
import sys
sys.path.insert(0, "/opt/trn_rl_repo")
import numpy as np

import os
import concourse.bacc as bacc
import concourse.tile as tile
from concourse import mybir

if os.environ.get("BASS_LDW_OPT") == "1":
    import concourse.bass_utils as _bu
    if not getattr(_bu, "_ldw_opt_patched", False):
        _orig_run_command = _bu.run_command

        def _run_command_ldw(argv, **kwargs):
            argv = ["--enable-ldw-opt=true" if a == "--enable-ldw-opt=false" else a
                    for a in argv]
            return _orig_run_command(argv, **kwargs)

        _bu.run_command = _run_command_ldw
        _bu._ldw_opt_patched = True

F32 = mybir.dt.float32
FP16 = mybir.dt.float16
SIG = mybir.ActivationFunctionType.Sigmoid
TANH = mybir.ActivationFunctionType.Tanh
MUL = mybir.AluOpType.mult
ADD = mybir.AluOpType.add
SUB = mybir.AluOpType.subtract

I_SIZE, H_SIZE = 256, 512
B_FULL, S_FULL = 64, 512
N_CORES = 8
BL = 32                      # batch rows per stream
W_WARM = 20                  # warmup steps for chunks > 0
N_CHUNK = 6                  # seq chunks per direction
NS = N_CHUNK // 2            # streams per core (2 dirs x 2 shards x N_CHUNK / 8)


def _chunk_geometry(S=S_FULL, n_chunk=N_CHUNK, w=W_WARM):
    """T, [(t0, real_lo, real_hi)] per chunk. real_lo/hi are step indices
    within the chunk's local [0, T) window; global t = t0 + local."""
    L = (S - w) // n_chunk          # real steps for chunks 1..n-1
    T = L + w                       # uniform per-unit step count
    assert L * n_chunk + w == S
    geo = []
    for k in range(n_chunk):
        t0 = 0 if k == 0 else L * k
        lo = 0 if k == 0 else w
        geo.append((t0, lo, T))
    return T, geo


T_STEPS, CHUNK_GEO = _chunk_geometry()


def _perm_cols():
    """new col' = 512*j + 128*g + k  maps from orig col = 512*g + 128*j + k."""
    p = np.empty(4 * H_SIZE, dtype=np.int64)
    for j in range(4):
        for g in range(4):
            for k in range(128):
                p[512 * j + 128 * g + k] = 512 * g + 128 * j + k
    return p


def build_program(T=T_STEPS, reps=1, timing=False, mode="full", tmode="pe"):
    """timing=True shrinks DRAM I/O (memset x, last-step y only) so wall-clock
    rep-differencing measures pure kernel time. mode="pe" keeps only the PE
    work (constant hT, no ACT/DVE chain) for engine bisection.
    tmode: "pe" = PE transpose + DVE copy; "dma" = DMA-XBAR transpose."""
    nc = bacc.Bacc("TRN2", target_bir_lowering=False, debug=False)

    if timing:
        d_x = nc.dram_tensor("x", [NS, 128, 64], FP16, kind="ExternalInput").ap()
        d_y = nc.dram_tensor("y", [NS, 2, 128, 128], FP16, kind="ExternalOutput").ap()
    else:
        d_x = nc.dram_tensor("x", [NS, 128, T * 64], FP16, kind="ExternalInput").ap()
        d_y = nc.dram_tensor("y", [NS, T, 128, 128], FP16, kind="ExternalOutput").ap()
    d_Wx = nc.dram_tensor("Wx", [2, 128, 2048], FP16, kind="ExternalInput").ap()
    d_Wh = nc.dram_tensor("Wh", [4, 128, 2048], FP16, kind="ExternalInput").ap()
    d_bias = nc.dram_tensor("bias", [1, 2048], FP16, kind="ExternalInput").ap()
    d_eyeT = nc.dram_tensor("eyeT", [128, 128], FP16, kind="ExternalInput").ap()

    with tile.TileContext(nc) as tc:
        with tc.tile_pool(name="pers", bufs=1) as pers, \
             tc.tile_pool(name="state", bufs=1) as st, \
             tc.tile_pool(name="work", bufs=4) as wk, \
             tc.tile_pool(name="ps", bufs=1, space="PSUM") as ps:

            wh_sb = pers.tile([128, 4, 2048], FP16, tag="wh")
            for c in range(4):
                nc.sync.dma_start(wh_sb[:, c, :], d_Wh[c, :, :])
            wx_sb = pers.tile([128, 2, 2048], FP16, tag="wx")
            for c in range(2):
                nc.sync.dma_start(wx_sb[:, c, :], d_Wx[c, :, :])
            bias_sb = pers.tile([1, 2048], FP16, tag="bias")
            nc.sync.dma_start(bias_sb[:], d_bias)
            eyeT_sb = pers.tile([128, 128], FP16, tag="eyeT")
            nc.sync.dma_start(eyeT_sb[:], d_eyeT)
            # one-hot stationary for bias inject: cols 0:32 = 1 (group 0),
            # rest 0 so the start=True MM writes+marks ALL 128 partitions.
            ones_sb = pers.tile([1, 128], FP16, tag="ones")
            nc.vector.memset(ones_sb[:], 0.0)
            nc.vector.memset(ones_sb[:, 0:32], 1.0)
            x_sb = pers.tile([128, NS, T * 64], FP16, tag="x")
            if timing:
                nc.vector.memset(x_sb[:], 0.02)
            else:
                for s in range(NS):
                    nc.sync.dma_start(x_sb[:, s, :], d_x[s, :, :])

            # persistent per-stream state (manual rotating slots)
            NG = 2
            gates_pp = [[ps.tile([128, 512], F32, tag=f"g{s}{i}", name=f"g{s}{i}")
                         for i in range(NG)] for s in range(NS)]
            tr_one = ps.tile([128, 128], FP16, tag="tr", name="trs") \
                if tmode == "pe" else None
            tr_ps = [tr_one for _ in range(NS)] if tmode == "pe" else None
            hT_pp = [[st.tile([128, 128], FP16, tag=f"hT{s}{i}", name=f"hT{s}{i}")
                      for i in range(2)] for s in range(NS)]
            c_pp = [[st.tile([128, 128], FP16, tag=f"c{s}{i}", name=f"c{s}{i}")
                     for i in range(2)] for s in range(NS)]
            for _rep in range(reps):
              for s in range(NS):
                nc.vector.memset(hT_pp[s][0][:], 0.0)
                nc.vector.memset(c_pp[s][0][:], 0.0)
                if mode.startswith("pe") and _rep == 0:
                    nc.vector.memset(hT_pp[s][1][:], 0.0)
                    nc.vector.memset(c_pp[s][1][:], 0.0)

              for t in range(T):
                for s in range(NS):
                    g = gates_pp[s][t % NG]
                    hT_prev = hT_pp[s][t % 2]
                    c_prev = c_pp[s][t % 2]
                    if mode in ("pe3", "pe4"):
                        # timing-only probes: same FLOP volume, alternative
                        # MM granularity (pe3: M=128 x1; pe4: M=64 x2)
                        nparts = {"pe3": 1, "pe4": 2}[mode]
                        mw = 128 // nparts
                        for cc in range(7):
                            src = wh_sb[:, cc % 4, :] if cc < 4 else wx_sb[:, cc % 2, :]
                            for j in range(nparts):
                                nc.tensor.matmul(
                                    g[mw * j:mw * (j + 1), :],
                                    hT_prev[:, mw * j:mw * (j + 1)],
                                    src[:, 512:1024],
                                    start=(cc == 0 and j == 0), stop=(cc == 6),
                                    skip_group_check=True,
                                    tile_position=(0, mw * j))
                        if t == T - 1 and _rep == reps - 1:
                            dmy = wk.tile([128, 128], FP16, tag=f"dmy{s}")
                            nc.scalar.activation(dmy[:], g[:, 0:128], SIG)
                            nc.sync.dma_start(d_y[s, 0], dmy[:])
                        continue
                    # bias inject: K=1 broadcast. j=0 spans all 128 partitions
                    # (zeros outside group 0) so start=True marks the whole
                    # bank written; j=1..3 then accumulate their bias.
                    nc.tensor.matmul(
                        g[:, :],
                        ones_sb[0:1, :],
                        bias_sb[0:1, 0:512],
                        start=True, stop=False, skip_group_check=True,
                        tile_position=(0, 0))
                    for j in range(1, 4):
                        nc.tensor.matmul(
                            g[32 * j:32 * (j + 1), :],
                            ones_sb[0:1, 0:32],
                            bias_sb[0:1, 512 * j:512 * (j + 1)],
                            start=False, stop=False, skip_group_check=True,
                            tile_position=(0, 32 * j))
                    # x chunks (independent of recurrence -> issue early)
                    for c in range(2):
                        xs = x_sb[:, s, 64 * t + 32 * c: 64 * t + 32 * (c + 1)]
                        for j in range(4):
                            nc.tensor.matmul(
                                g[32 * j:32 * (j + 1), :],
                                xs,
                                wx_sb[:, c, 512 * j:512 * (j + 1)],
                                start=False, stop=False, skip_group_check=True,
                                tile_position=(0, 32 * j))
                    # h chunks
                    for c in range(4):
                        hs = hT_prev[:, 32 * c:32 * (c + 1)]
                        for j in range(4):
                            nc.tensor.matmul(
                                g[32 * j:32 * (j + 1), :],
                                hs,
                                wh_sb[:, c, 512 * j:512 * (j + 1)],
                                start=False, stop=(c == 3), skip_group_check=True,
                                tile_position=(0, 32 * j))
                    if mode == "pe":
                        if t == T - 1 and _rep == reps - 1:
                            dmy = wk.tile([128, 128], FP16, tag=f"dmy{s}")
                            nc.scalar.activation(dmy[:], g[:, 0:128], SIG)
                            nc.gpsimd.dma_start(d_y[s, 0], dmy[:])
                        continue
                    # one sigmoid for all four gates ([f,i,o,2g] pre-acts)
                    sg = wk.tile([128, 512], FP16, tag=f"sg{s}")
                    nc.scalar.activation(sg[:], g[:, :], SIG)
                    # cell update (fp16): c' = sf*c + si*(2*sg2 - 1)
                    m_t = wk.tile([128, 128], FP16, tag=f"m{s}")
                    nc.vector.tensor_mul(m_t[:], sg[:, 0:128], c_prev[:])
                    u_t = wk.tile([128, 128], FP16, tag=f"u{s}")
                    nc.vector.scalar_tensor_tensor(
                        u_t[:], sg[:, 384:512], 0.5, sg[:, 128:256], SUB, MUL)
                    c_new = c_pp[s][(t + 1) % 2]
                    nc.vector.scalar_tensor_tensor(
                        c_new[:], u_t[:], 2.0, m_t[:], MUL, ADD)
                    # h' = h/2 = (sigmoid(2c) - 0.5) * so  (tanh avoided so the
                    # ACT table set never switches; Wh is pre-scaled x2 and the
                    # final x2 is absorbed in host-side assembly)
                    tc_t = wk.tile([128, 128], FP16, tag=f"tc{s}")
                    nc.scalar.activation(tc_t[:], c_new[:], SIG, scale=2.0)
                    h_t = wk.tile([128, 128], FP16, tag=f"h{s}")
                    nc.vector.scalar_tensor_tensor(
                        h_t[:], tc_t[:], 0.5, sg[:, 256:384], SUB, MUL)
                    # transpose h for next step's stationary
                    if t + 1 < T:
                        if tmode == "dma":
                            nc.sync.dma_start_transpose(
                                hT_pp[s][(t + 1) % 2][:], h_t[:])
                        else:
                            nc.tensor.transpose(tr_ps[s][:], h_t[:], eyeT_sb[:])
                            nc.vector.tensor_copy(
                                hT_pp[s][(t + 1) % 2][:], tr_ps[s][:])
                    nc.sync.dma_start(d_y[s, t % 2 if timing else t], h_t[:])

    nc.compile()
    return nc


def _prep_weights(W, b):
    perm = _perm_cols()
    Wp = np.asarray(W)[:, perm].astype(np.float32).copy()
    bp = np.asarray(b)[perm].astype(np.float32).copy()
    # scale C-gate (g=3) columns x2: tanh(g) = 2*sigmoid(2g) - 1
    for j in range(4):
        Wp[:, 512 * j + 384: 512 * j + 512] *= 2.0
        bp[512 * j + 384: 512 * j + 512] *= 2.0
    # device h is stored as h/2 -> compensate in the recurrent weights
    Wp[I_SIZE:, :] *= 2.0
    Wx = np.ascontiguousarray(Wp[:I_SIZE]).reshape(2, 128, 2048).astype(np.float16)
    Wh = np.ascontiguousarray(Wp[I_SIZE:]).reshape(4, 128, 2048).astype(np.float16)
    bias = bp.reshape(1, 2048).astype(np.float16)
    return Wx, Wh, bias


def _prep_x_window(x_slice, t0, T):
    """x_slice [BL, S, I] f32 (already time-reversed for bwd).
    Returns [128, T*64] fp16 with layout [p, t*64 + 32*c + b]."""
    xx = np.asarray(x_slice[:, t0:t0 + T, :])          # [32, T, 256]
    arr = xx.reshape(BL, T, 2, 128).transpose(3, 1, 2, 0)  # [128, T, 2, 32]
    return np.ascontiguousarray(arr).reshape(128, T * 64).astype(np.float16)


# unit table: core -> (dir, shard, chunk tuple)
def _core_units():
    units = []
    for d in range(2):
        for sh in range(2):
            for cp in range(2):
                units.append((d, sh, tuple(range(NS * cp, NS * (cp + 1)))))
    return units   # 8 cores


def make_in_maps(inputs, W_f, b_f, W_b, b_b, T=T_STEPS):
    Wx_f, Wh_f, bias_f = _prep_weights(W_f, b_f)
    Wx_b, Wh_b, bias_b = _prep_weights(W_b, b_b)
    eyeT = np.eye(128, dtype=np.float16)
    x = np.asarray(inputs, dtype=np.float32)
    in_maps = []
    for (d, sh, chunks) in _core_units():
        xs = x[BL * sh: BL * (sh + 1)]
        if d == 1:
            xs = xs[:, ::-1, :]
        xw = np.empty((NS, 128, T * 64), np.float16)
        for i, k in enumerate(chunks):
            t0, _, _ = CHUNK_GEO[k]
            xw[i] = _prep_x_window(xs, t0, T)
        Wx, Wh, bias = (Wx_f, Wh_f, bias_f) if d == 0 else (Wx_b, Wh_b, bias_b)
        in_maps.append({"x": xw, "Wx": Wx, "Wh": Wh, "bias": bias, "eyeT": eyeT})
    return in_maps


def assemble_output(results, S=S_FULL, B=B_FULL, T=T_STEPS):
    out = np.zeros((2, B, S, H_SIZE), np.float32)
    for core, (d, sh, chunks) in enumerate(_core_units()):
        y = np.asarray(results[core]["y"], np.float32)   # [2, T, 128, 128]
        for i, k in enumerate(chunks):
            t0, lo, hi = CHUNK_GEO[k]
            yc = y[i, lo:hi]                              # [n, 128, 128]
            n = hi - lo
            hc = yc.reshape(n, 4, 32, 128).transpose(2, 0, 1, 3).reshape(BL, n, H_SIZE)
            if d == 0:
                out[0, BL * sh: BL * (sh + 1), t0 + lo: t0 + hi] = hc
            else:
                # bwd: reversed time axis; global reversed t = t0+local
                rev_lo, rev_hi = t0 + lo, t0 + hi
                out[1, BL * sh: BL * (sh + 1), S - rev_hi: S - rev_lo] = hc[:, ::-1]
    # device y holds h/2, so (h_f + h_b)/2 = y_f + y_b
    return out[0] + out[1]


_NC_CACHE = {}


def _make_runner(nc, n_cores):
    """Reusable jitted SPMD runner (axon/PJRT path) — builds the sharded
    jit once so repeated kernel() calls skip retrace + NEFF recompile."""
    import jax
    from jax.experimental.shard_map import shard_map
    from jax.sharding import Mesh, PartitionSpec
    from concourse import bass2jax

    bass2jax.install_neuronx_cc_hook()
    partition_name = nc.partition_id_tensor.name if nc.partition_id_tensor else None
    in_names, out_names, out_avals, zero_outs = [], [], [], []
    for alloc in nc.m.functions[0].allocations:
        if not isinstance(alloc, mybir.MemoryLocationSet):
            continue
        name = alloc.memorylocations[0].name
        if alloc.kind == "ExternalInput":
            if name != partition_name:
                in_names.append(name)
        elif alloc.kind == "ExternalOutput":
            shape = tuple(alloc.tensor_shape)
            dtype = mybir.dt.np(alloc.dtype)
            out_names.append(name)
            out_avals.append(jax.core.ShapedArray(shape, dtype))
            zero_outs.append(np.zeros(shape, dtype))
    n_params = len(in_names)
    n_outs = len(out_avals)
    all_in = list(in_names) + list(out_names)
    if partition_name is not None:
        all_in.append(partition_name)
    donate = tuple(range(n_params, n_params + n_outs))

    def _body(*args):
        operands = list(args)
        if partition_name is not None:
            operands.append(bass2jax.partition_id_tensor())
        outs = bass2jax._bass_exec_p.bind(
            *operands,
            out_avals=tuple(out_avals),
            in_names=tuple(all_in),
            out_names=tuple(out_names),
            lowering_input_output_aliases=(),
            sim_require_finite=True,
            sim_require_nnan=True,
            nc=nc,
        )
        return tuple(outs)

    devices = jax.devices()[:n_cores]
    assert len(devices) == n_cores
    mesh = Mesh(np.asarray(devices), ("core",))
    in_specs = (PartitionSpec("core"),) * (n_params + n_outs)
    out_specs = (PartitionSpec("core"),) * n_outs
    sharded = jax.jit(
        shard_map(_body, mesh=mesh, in_specs=in_specs,
                  out_specs=out_specs, check_rep=False),
        donate_argnums=donate,
        keep_unused=True,
    )

    def run(in_maps):
        per_core = [[np.asarray(m[name]) for name in in_names] for m in in_maps]
        concat_in = [
            np.concatenate([per_core[c][i] for c in range(n_cores)], axis=0)
            for i in range(n_params)
        ]
        concat_zeros = [
            np.zeros((n_cores * z.shape[0], *z.shape[1:]), z.dtype)
            for z in zero_outs
        ]
        out_arrs = sharded(*concat_in, *concat_zeros)
        return [
            {
                name: np.asarray(out_arrs[i]).reshape(n_cores, *out_avals[i].shape)[c]
                for i, name in enumerate(out_names)
            }
            for c in range(n_cores)
        ]

    return run


def kernel(inputs, W_f, b_f, W_b, b_b):
    inputs = np.asarray(inputs, dtype=np.float32)
    if "run" not in _NC_CACHE:
        try:
            _NC_CACHE["run"] = _make_runner(build_program(), N_CORES)
        except Exception:
            _NC_CACHE["run"] = None
    in_maps = make_in_maps(inputs, W_f, b_f, W_b, b_b)
    run = _NC_CACHE.get("run")
    if run is not None:
        results = run(in_maps)
    else:
        from concourse.bass_utils import run_bass_kernel_spmd
        if "prog" not in _NC_CACHE:
            _NC_CACHE["prog"] = build_program()
        results = run_bass_kernel_spmd(
            _NC_CACHE["prog"], in_maps, core_ids=list(range(N_CORES))).results
    return assemble_output(results)
